# revision 18
# baseline (speedup 1.0000x reference)
"""GAT-style masked-attention kernel for Trainium2, sharded over 8 NeuronCores.

Math (per head h):
    Wh  = h @ Ws[h]                       [N, 64]
    e   = lrelu(Wh@a1 + (Wh@a2)^T, 0.2)   [N, N]
    att = softmax(where(mask, e, -inf))   row-wise
    out = elu(att @ Wh) ; concat heads ; log_softmax rows

Sharding: row-shard the N=8000 queries across 8 cores (1000 rows each,
padded to 1024). Keys j live on SBUF partitions (8192 padded, 64 tiles
of 128); queries i on the free dim. Per (j-tile, head):
    v = Prelu(Wh1_bcast * 1 + Wh2_j, alpha=0.2)     (ACT, per-part bias)
    t = Exp(v)                                      (ACT)
    S = t * maskT                                   (DVE, bf16 mask)
    acc[65, i] += [Wh | 1]^T_j @ S                  (PE, numer + denom)
Epilogue: PE-transpose, normalize by denom, ELU, row log_softmax.

Host does layout-only prep: assemble/transpose/pad/cast the block mask,
transpose h, split a_param. All FLOPs of the problem run on device.
"""

import sys

sys.path.insert(0, "/opt/trn_rl_repo")

import numpy as np
import ml_dtypes

import concourse.bacc as bacc
import concourse.tile as tile
from concourse import mybir
from concourse.bass_utils import run_bass_kernel_spmd
from concourse.masks import make_identity

dt = mybir.dt
AF = mybir.ActivationFunctionType
ALU = mybir.AluOpType

N1, N2, N3 = 3000, 2500, 2500
N = N1 + N2 + N3          # 8000
IN_F, OUT_F, HEADS = 128, 64, 2
ALPHA = 0.2

NCORES = 8
J = 8192                  # padded key count
JT = J // 128             # 64 j-tiles
IPR = N // NCORES         # 1000 real query rows per core
IPC = 1024                # padded query rows per core
IC = IPC // 512           # moving-operand chunks


def _build_program(repeat=1, empty=False, minimal=False):
    import time
    _t0 = time.time()
    nc = bacc.Bacc("TRN2", target_bir_lowering=False, debug=False,
                   num_devices=NCORES)

    maskT_d = nc.dram_tensor("maskT", [J, IPC], dt.bfloat16, kind="ExternalInput")
    hT_d = nc.dram_tensor("hT", [IN_F, J], dt.float32, kind="ExternalInput")
    hTo_d = nc.dram_tensor("hTo", [IN_F, IPC], dt.float32, kind="ExternalInput")
    ws_d = [nc.dram_tensor(f"ws{h}", [IN_F, OUT_F], dt.float32, kind="ExternalInput")
            for h in range(HEADS)]
    wsT_d = [nc.dram_tensor(f"wsT{h}", [OUT_F, IN_F], dt.float32, kind="ExternalInput")
             for h in range(HEADS)]
    ac_d = [nc.dram_tensor(f"ac{h}", [OUT_F, 2], dt.float32, kind="ExternalInput")
            for h in range(HEADS)]
    out_d = nc.dram_tensor("out", [IPC, HEADS * OUT_F], dt.float32,
                           kind="ExternalOutput")

    if minimal:
        with tile.TileContext(nc) as tc:
            with tc.tile_pool(name="mn", bufs=1) as mn:
                z = mn.tile([128, HEADS * OUT_F], dt.float32)
                nc.vector.memset(z, 0.0)
                for it in range(IPC // 128):
                    nc.sync.dma_start(out=out_d.ap()[it * 128:(it + 1) * 128, :],
                                      in_=z)
        nc.compile()
        return nc

    with tile.TileContext(nc) as tc:
        with tc.tile_pool(name="const", bufs=1) as const, \
             tc.tile_pool(name="prol", bufs=1) as prol, \
             tc.tile_pool(name="waugp", bufs=1) as waugp, \
             tc.tile_pool(name="mpool", bufs=4) as mpool, \
             tc.tile_pool(name="vpool", bufs=2) as vpool, \
             tc.tile_pool(name="tpool", bufs=2) as tpool, \
             tc.tile_pool(name="spool", bufs=3) as spool, \
             tc.tile_pool(name="epool", bufs=2) as epool, \
             tc.tile_pool(name="psA", bufs=1, space="PSUM") as psA, \
             tc.tile_pool(name="psB", bufs=2, space="PSUM") as psB, \
             tc.tile_pool(name="drp", bufs=1, space="DRAM") as drp:

            # ---------------- prologue: projections -----------------
            hT = const.tile([IN_F, J], dt.float32)
            nc.sync.dma_start(out=hT, in_=hT_d.ap())
            hTo = const.tile([IN_F, IPC], dt.float32)
            nc.sync.dma_start(out=hTo, in_=hTo_d.ap())
            ident = const.tile([128, 128], dt.float32)
            make_identity(nc, ident)

            ws_sb, wsT_sb, ac_sb = [], [], []
            for h in range(HEADS):
                w = const.tile([IN_F, OUT_F], dt.float32, tag=f"ws{h}")
                nc.sync.dma_start(out=w, in_=ws_d[h].ap())
                ws_sb.append(w)
                wt_ = const.tile([OUT_F, IN_F], dt.float32, tag=f"wsT{h}")
                nc.sync.dma_start(out=wt_, in_=wsT_d[h].ap())
                wsT_sb.append(wt_)
                a = const.tile([OUT_F, 2], dt.float32, tag=f"ac{h}")
                nc.gpsimd.dma_start(out=a, in_=ac_d[h].ap())
                ac_sb.append(a)

            waug, wh1b, wh2c = [], [], []
            for h in range(HEADS):
                # Waug[j, 0:64] = Wh rows, Waug[j, 64] = 1.0 (denominator)
                wa = waugp.tile([128, JT * (OUT_F + 1)], dt.float32, tag=f"waug{h}")
                nc.vector.memset(wa, 1.0)
                for jt in range(JT):
                    pw = psB.tile([128, OUT_F], dt.float32, tag="psb")
                    nc.tensor.matmul(pw, lhsT=hT[:, jt * 128:(jt + 1) * 128],
                                     rhs=ws_sb[h], start=True, stop=True)
                    nc.vector.tensor_copy(
                        wa[:, jt * 65: jt * 65 + OUT_F], pw)
                waug.append(wa)

                # g = Ws @ [a1|a2]  [128, 2]; then Wh12 rows = g^T @ hT  [2, J]
                gp = psB.tile([IN_F, 2], dt.float32, tag="psb")
                nc.tensor.matmul(gp, lhsT=wsT_sb[h], rhs=ac_sb[h],
                                 start=True, stop=True)
                g = prol.tile([IN_F, 2], dt.float32, tag=f"g{h}")
                nc.vector.tensor_copy(g, gp)
                w12_dr = drp.tile([2, J], dt.float32, tag=f"w12d{h}")
                for c in range(J // 512):
                    pr = psB.tile([2, 512], dt.float32, tag="psb")
                    nc.tensor.matmul(pr, lhsT=g,
                                     rhs=hT[:, c * 512:(c + 1) * 512],
                                     start=True, stop=True)
                    sr = epool.tile([2, 512], dt.float32, tag="sr")
                    nc.vector.tensor_copy(sr, pr)
                    nc.gpsimd.dma_start(out=w12_dr[:, c * 512:(c + 1) * 512],
                                        in_=sr)

                # own-row Wh1 [1, IPC]
                w1o_dr = drp.tile([1, IPC], dt.float32, tag=f"w1od{h}")
                for c in range(IPC // 512):
                    pr2 = psB.tile([1, 512], dt.float32, tag="psb")
                    nc.tensor.matmul(pr2, lhsT=g[:, 0:1],
                                     rhs=hTo[:, c * 512:(c + 1) * 512],
                                     start=True, stop=True)
                    sr2 = epool.tile([1, 512], dt.float32, tag="sr2")
                    nc.vector.tensor_copy(sr2, pr2)
                    nc.gpsimd.dma_start(out=w1o_dr[:, c * 512:(c + 1) * 512],
                                        in_=sr2)

                # broadcast own Wh1 across partitions; Wh2 as per-tile columns
                b1 = const.tile([128, IPC], dt.float32, tag=f"wh1b{h}")
                nc.gpsimd.dma_start(out=b1, in_=w1o_dr[0:1, :].to_broadcast([128, IPC]))
                wh1b.append(b1)
                c2 = const.tile([128, JT], dt.float32, tag=f"wh2c{h}")
                nc.gpsimd.dma_start(
                    out=c2, in_=w12_dr[1, :].rearrange("(t p) -> p t", p=128))
                wh2c.append(c2)

            # ---------------- main loop -----------------
            acc = [[psA.tile([OUT_F + 1, 512], dt.float32, tag=f"acc{h}_{ic}",
                             name=f"acc{h}_{ic}")
                    for ic in range(IC)] for h in range(HEADS)]
            for _rep in range(0 if empty else repeat):
                for jt in range(JT):
                    mt = mpool.tile([128, IPC], dt.bfloat16, name="mt")
                    nc.sync.dma_start(out=mt,
                                      in_=maskT_d.ap()[jt * 128:(jt + 1) * 128, :])
                    for h in range(HEADS):
                        vt = vpool.tile([128, IPC], dt.float32, name="vt")
                        nc.scalar.activation(vt, wh1b[h], AF.Prelu,
                                             bias=wh2c[h][:, jt:jt + 1],
                                             scale=1.0, alpha=ALPHA)
                        tt = tpool.tile([128, IPC], dt.float32, name="tt")
                        nc.scalar.activation(tt, vt, AF.Exp)
                        st = spool.tile([128, IPC], dt.float32, name="st")
                        nc.vector.tensor_tensor(out=st, in0=tt, in1=mt, op=ALU.mult)
                        for ic in range(IC):
                            nc.tensor.matmul(
                                acc[h][ic],
                                lhsT=waug[h][:, jt * 65:(jt + 1) * 65],
                                rhs=st[:, ic * 512:(ic + 1) * 512],
                                start=(jt == 0), stop=(jt == JT - 1))
            if empty:
                for h in range(HEADS):
                    for ic in range(IC):
                        nc.vector.memset(acc[h][ic], 1.0)

            # ---------------- epilogue -----------------
            mids = []
            for h in range(HEADS):
                mid = prol.tile([OUT_F + 1, IPC], dt.float32, tag=f"mid{h}")
                for ic in range(IC):
                    nc.vector.tensor_copy(mid[:, ic * 512:(ic + 1) * 512],
                                          acc[h][ic])
                mids.append(mid)

            for it in range(IPC // 128):
                x2 = epool.tile([128, 2 * 65], dt.float32, tag="x2")
                for h in range(HEADS):
                    ptr = psB.tile([128, OUT_F + 1], dt.float32, tag="psb")
                    nc.tensor.transpose(ptr, mids[h][:, it * 128:(it + 1) * 128],
                                        ident[0:OUT_F + 1, 0:OUT_F + 1])
                    nc.vector.tensor_copy(x2[:, h * 65:(h + 1) * 65], ptr)
                # att-normalize: x = numer / denom
                xe = epool.tile([128, 128], dt.float32, tag="xe")
                for h in range(HEADS):
                    rc = epool.tile([128, 1], dt.float32, tag=f"rc{h}")
                    nc.vector.reciprocal(rc, x2[:, h * 65 + 64: h * 65 + 65])
                    nc.vector.tensor_scalar(
                        out=xe[:, h * 64:(h + 1) * 64],
                        in0=x2[:, h * 65: h * 65 + 64],
                        scalar1=rc[:, 0:1], scalar2=None, op0=ALU.mult)
                # ELU: pos(x) + exp(min(x,0)) - 1
                neg = epool.tile([128, 128], dt.float32, tag="neg")
                nc.vector.tensor_scalar(out=neg, in0=xe, scalar1=0.0,
                                        scalar2=None, op0=ALU.min)
                pos = epool.tile([128, 128], dt.float32, tag="pos")
                nc.vector.tensor_scalar(out=pos, in0=xe, scalar1=0.0,
                                        scalar2=None, op0=ALU.max)
                en = epool.tile([128, 128], dt.float32, tag="en")
                nc.scalar.activation(en, neg, AF.Exp)
                enm1 = epool.tile([128, 128], dt.float32, tag="enm1")
                nc.vector.tensor_scalar(out=enm1, in0=en, scalar1=1.0,
                                        scalar2=None, op0=ALU.subtract)
                elu = epool.tile([128, 128], dt.float32, tag="elu")
                nc.vector.tensor_tensor(out=elu, in0=pos, in1=enm1, op=ALU.add)
                # log_softmax over the 128 features
                mx = epool.tile([128, 1], dt.float32, tag="mx")
                nc.vector.tensor_reduce(out=mx, in_=elu,
                                        axis=mybir.AxisListType.X, op=ALU.max)
                negm = epool.tile([128, 1], dt.float32, tag="negm")
                nc.vector.tensor_scalar(out=negm, in0=mx, scalar1=-1.0,
                                        scalar2=None, op0=ALU.mult)
                ex2 = epool.tile([128, 128], dt.float32, tag="ex2")
                ssum = epool.tile([128, 1], dt.float32, tag="ssum")
                nc.scalar.activation(ex2, elu, AF.Exp, bias=negm[:, 0:1],
                                     accum_out=ssum)
                lns = epool.tile([128, 1], dt.float32, tag="lns")
                nc.scalar.activation(lns, ssum, AF.Ln)
                cc = epool.tile([128, 1], dt.float32, tag="cc")
                nc.vector.tensor_tensor(out=cc, in0=mx, in1=lns, op=ALU.add)
                outt = epool.tile([128, 128], dt.float32, tag="outt")
                nc.vector.tensor_scalar(out=outt, in0=elu, scalar1=cc[:, 0:1],
                                        scalar2=None, op0=ALU.subtract)
                nc.sync.dma_start(out=out_d.ap()[it * 128:(it + 1) * 128, :],
                                  in_=outt)

    print(f"[kernel] trace done {time.time()-_t0:.1f}s", flush=True)
    nc.compile()
    print(f"[kernel] bacc compile done {time.time()-_t0:.1f}s", flush=True)
    return nc


_NC_CACHE = None


def _get_program():
    global _NC_CACHE
    if _NC_CACHE is None:
        _NC_CACHE = _build_program()
    return _NC_CACHE


def kernel(h, A1, A2, A3, A12, A13, A23, A21, A31, A32, Ws, a_param):
    h = np.asarray(h, np.float32)
    Ws = np.asarray(Ws, np.float32)
    a_param = np.asarray(a_param, np.float32)

    # --- host layout prep (no math): mask assemble + transpose + cast ---
    mask = np.empty((N, N), np.bool_)
    rows = [np.asarray(A1), np.asarray(A12), np.asarray(A13),
            np.asarray(A21), np.asarray(A2), np.asarray(A23),
            np.asarray(A31), np.asarray(A32), np.asarray(A3)]
    ofs = [0, N1, N1 + N2, N]
    for r in range(3):
        for c in range(3):
            mask[ofs[r]:ofs[r + 1], ofs[c]:ofs[c + 1]] = rows[r * 3 + c] > 0
    maskT = mask.T  # [j, i]

    hT = np.zeros((IN_F, J), np.float32)
    hT[:, :N] = h.T

    in_maps = []
    for c in range(NCORES):
        mc = np.ones((J, IPC), ml_dtypes.bfloat16)
        mc[:N, :IPR] = maskT[:, c * IPR:(c + 1) * IPR].astype(ml_dtypes.bfloat16)
        mc[N:, :] = 0
        hTo = np.zeros((IN_F, IPC), np.float32)
        hTo[:, :IPR] = h.T[:, c * IPR:(c + 1) * IPR]
        m = {"maskT": mc, "hT": hT, "hTo": hTo}
        for hh in range(HEADS):
            m[f"ws{hh}"] = np.ascontiguousarray(Ws[hh])
            m[f"wsT{hh}"] = np.ascontiguousarray(Ws[hh].T)
            m[f"ac{hh}"] = np.ascontiguousarray(
                np.stack([a_param[hh, :OUT_F, 0], a_param[hh, OUT_F:, 0]], axis=1))
        in_maps.append(m)

    import time
    nc = _get_program()
    _t0 = time.time()
    res = run_bass_kernel_spmd(nc, in_maps, list(range(NCORES)))
    print(f"[kernel] spmd run (incl neff compile on first call) "
          f"{time.time()-_t0:.1f}s", flush=True)
    out_full = np.concatenate(
        [res.results[c]["out"][:IPR] for c in range(NCORES)], axis=0)

    x3 = out_full[N1 + N2:]
    x4 = out_full[:N1]
    x5 = out_full[N1:N1 + N2]
    return (x3, x4, x5)
